# revision 6
# baseline (speedup 1.0000x reference)
"""Batch-all triplet loss on 8 Trainium2 NeuronCores (Bass/Tile).

Math: with d = pairwise euclidean distance matrix of the B embeddings,
  loss = sum_{i,j,k valid} relu(d[i,j] - d[i,k] + margin) / (#positive + eps)
valid <=> i != j, labels[i] == labels[j], labels[i] != labels[k]
(the other distinctness constraints are implied by the label ones).

Sharding: anchors are grouped by class; each core hosts 2 classes in two
row blocks ([0:63) and [64:127)); rows 63/127 carry the per-class column
masks (-SHIFT on valid columns, +BIG on own-class ones), folded into the
replication matmul as a 2nd one in each stationary column (2-hot).

Device pipeline per core:
  warmup: ~14 dummy matmuls while input DMAs stream, so the PE's HAM
    governor ramps 1.2 -> 2.4 GHz before the real matmuls issue.
  prep: combo = [gathered anchors(128) | X^T(640)] bf16, K-packed.
    g = X_a.X^T - 0.5*sq_j (bf16 K-chunks + a K=2 hi/lo norm row pair)
    -> ACT Relu(-2g + sq_a) -> ACT Sqrt -> rhs_c[:, 0:B] (bf16 distances).
    Mask rows 63/127 (incl. zeroed bias cols) come from a tiny host DMA.
    A partner matmul against the anchor block gives bias hi/lo bf16 cols
    rhs_c[:, B:B+2T] (bias = d_pos + margin - SHIFT, hi/lo split keeps the
    pair threshold ~f32 exact).
  pair loop, NT tiles of 128 (anchor,positive) pairs:
  - PE 2-hot matmul (anchor row + its class's mask row) replicates
    rep[p, :] = d_masked[anchor(p), :] | bias cols, into PSUM.
  - DVE stt: sel one-hot picks the pair's own bias -> bias_p [128,1] f32.
  - ACT: o1 = Relu(-rep + bias_p) -> SBUF bf16 (the relu'd triplet values).
  - every 2 tiles, DVE on the packed bf16 o1 (4x perf mode):
    tensor_scalar(not_equal 0, accum) -> positive count;
    tensor_reduce(add) -> per-tile sums.
  - invalid negatives see +BIG (contribute 0); padded pairs have all-zero
    rows and zero bias (contribute 0 to both sums).
  tail: free-dim reduce -> [128, 2], gpsimd partition_all_reduce -> one
    [1, 2] (sum, count) DMA'd out; host combines the 8 cores.
"""

import numpy as np

import bass_rust
import concourse.bass as bass
import concourse.tile as tile
from concourse import mybir
from concourse import bass_isa
from concourse.bass_utils import run_bass_kernel_spmd

N_CORES = 8
D_MODEL = 512
B_TOTAL = 640
MARGIN = 0.3
EPS = 1e-8
RB = 64          # rows per class block; rows RB-1 / 2*RB-1 are mask rows
CMAX = RB - 1    # max class size the device path supports
BIG = 1.0e30
SHIFT = 32.0     # d values live in ~[27.6, 37]; recentring helps bf16
F32 = mybir.dt.float32
BF16 = mybir.dt.bfloat16
NP_BF16 = mybir.dt.np(mybir.dt.bfloat16)

_PROGRAM_CACHE: dict = {}


def _split_multi_waits(nc):
    """This toolchain's walrus codegen supports only ONE sync-wait per
    instruction; Tile can emit several. Move the extra waits onto
    same-engine NoOps inserted immediately before the instruction."""
    for func in nc.m.functions:
        for block in func.blocks:
            out = []
            for inst in block.instructions:
                si = inst.sync_info
                waits = list(si.on_wait) if si else []
                if len(waits) > 1:
                    for j, w in enumerate(waits[:-1]):
                        nop = mybir.InstNoOp(
                            name=f"{inst.name}-wsplit{j}", ins=[], outs=[]
                        )
                        nop.engine = inst.engine
                        nop.sync_info = bass_rust.SyncInfo(on_wait=[w], on_update=[])
                        out.append(nop)
                    inst.sync_info = bass_rust.SyncInfo(
                        on_wait=[waits[-1]], on_update=list(si.on_update)
                    )
                out.append(inst)
            block.instructions = out


def _build_program(B: int, D: int, T: int, NT: int):
    """One SPMD program for all 8 cores; per-core behavior is data-driven."""
    nc = bass.Bass()

    KC = D // 128     # K chunks of the matmul contraction
    W = 128 + B       # combo cols: [0:128) gathered anchors, [128:) X^T
    WR = B + 2 * T    # replicated tile: negatives + bias hi + lo columns

    # packed [128, KC, W]: partition p, chunk c -> K-row c*128+p
    combo = nc.declare_dram_parameter("combo", [128, KC * W], BF16, isOutput=False)
    aug = nc.declare_dram_parameter("aug", [2, W], BF16, isOutput=False)
    sq_a = nc.declare_dram_parameter("sq_a", [128, 1], F32, isOutput=False)
    maskrow = nc.declare_dram_parameter("maskrow", [2, WR], BF16, isOutput=False)
    onehot = nc.declare_dram_parameter("onehot", [128, NT * 128], BF16, isOutput=False)
    sel = nc.declare_dram_parameter("sel", [128, NT * 2 * T], BF16, isOutput=False)
    out_d = nc.declare_dram_parameter("out", [1, 2], F32, isOutput=True)

    NTC = (NT + 1) // 2  # count/reduce instruction pairs (2 tiles each)

    with tile.TileContext(nc) as tc:
        with (
            tc.tile_pool(name="const", bufs=1) as const,
            tc.tile_pool(name="work", bufs=1) as work,
        ):
            # ---- PE warmup source + ACT table preload -------------------
            warm_src = const.tile([128, 384], BF16)
            nc.vector.memset(warm_src, 0.0)
            warm = const.tile([1, 8], F32)
            nc.vector.memset(warm, 1.0)
            nc.scalar.activation(out=warm, in_=warm,
                                 func=mybir.ActivationFunctionType.Relu)
            nc.scalar.activation(out=warm, in_=warm,
                                 func=mybir.ActivationFunctionType.Sqrt)

            # ---- input DMAs, spread across engine queues -----------------
            ktile = const.tile([128, KC, W], BF16)
            combo3 = combo[:, :].rearrange("p (c w) -> p c w", c=KC)
            for ki in range(KC):
                nc.sync.dma_start(out=ktile[:, ki, :], in_=combo3[:, ki, :])
            taug = const.tile([2, W], BF16)
            nc.scalar.dma_start(out=taug, in_=aug[:, :])
            sqa = const.tile([128, 1], F32)
            nc.scalar.dma_start(out=sqa, in_=sq_a[:, :])
            t_oh = const.tile([128, NT * 128], BF16)
            ohq = (NT + 3) // 4 * 128
            for c in range(4):
                c0, c1 = c * ohq, min((c + 1) * ohq, NT * 128)
                if c0 < c1:
                    nc.gpsimd.dma_start(out=t_oh[:, c0:c1], in_=onehot[:, c0:c1])
            t_sel = const.tile([128, NT * 2 * T], BF16)
            sq2 = (NT + 1) // 2 * 2 * T
            for c in range(2):
                c0, c1 = c * sq2, min((c + 1) * sq2, NT * 2 * T)
                if c0 < c1:
                    nc.gpsimd.dma_start(out=t_sel[:, c0:c1], in_=sel[:, c0:c1])

            ones2 = const.tile([2, 128], BF16)
            nc.vector.memset(ones2, 1.0)

            rhs_c = work.tile([128, WR], BF16)  # [d_in | bias hi | bias lo]

            with tc.tile_pool(name="psum_warm", bufs=1, space="PSUM") as psw:
                wp = psw.tile([128, 384], F32)
                for _ in range(14):
                    nc.tensor.matmul(wp, warm_src[:, 0:128], warm_src,
                                     start=True, stop=True)

            with tc.tile_pool(name="psum_prep", bufs=1, space="PSUM") as psp:
                # ---- distance rows: PSUM g = dot(x_a, x_j) - 0.5*sq_j ----
                # g col j <-> combo col 128+j
                g = psp.tile([128, B], F32)
                for n0, n1 in [(0, 512), (512, B)]:
                    for ki in range(KC):
                        nc.tensor.matmul(
                            g[:, n0:n1],
                            ktile[:, ki, 0:128],
                            ktile[:, ki, 128 + n0:128 + n1],
                            start=(ki == 0),
                            stop=False,
                        )
                    nc.tensor.matmul(
                        g[:, n0:n1], ones2, taug[:, 128 + n0:128 + n1],
                        start=False, stop=True,
                    )
                dsq = work.tile([128, B], F32)
                nc.scalar.activation(
                    out=dsq, in_=g, func=mybir.ActivationFunctionType.Relu,
                    bias=sqa, scale=-2.0,
                )
                nc.scalar.activation(
                    out=rhs_c[:, 0:B], in_=dsq,
                    func=mybir.ActivationFunctionType.Sqrt,
                )

                # ---- positive-pair distances: bias[r, t] -----------------
                pb = psp.tile([128, T], F32)
                for blk in range(2):
                    r0, r1 = blk * RB, (blk + 1) * RB
                    for ki in range(KC):
                        nc.tensor.matmul(
                            pb[r0:r1, :],
                            ktile[:, ki, r0:r1],
                            ktile[:, ki, r0:r0 + T],
                            start=(ki == 0),
                            stop=False,
                        )
                    nc.tensor.matmul(
                        pb[r0:r1, :], ones2[:, r0:r1], taug[:, r0:r0 + T],
                        start=False, stop=True,
                    )
                bsq = work.tile([128, T], F32)
                nc.scalar.activation(
                    out=bsq, in_=pb, func=mybir.ActivationFunctionType.Relu,
                    bias=sqa, scale=-2.0,
                )
                bd = work.tile([128, T], F32)
                nc.scalar.activation(
                    out=bd, in_=bsq, func=mybir.ActivationFunctionType.Sqrt,
                )
                # bias = d_pos + margin - 32, carried as bf16 hi + lo so
                # the pair threshold stays ~f32 exact
                bias_f = work.tile([128, T], F32)
                nc.vector.tensor_scalar(
                    out=bias_f, in0=bd,
                    scalar1=float(MARGIN - SHIFT), scalar2=None,
                    op0=mybir.AluOpType.add,
                )
                nc.vector.tensor_copy(rhs_c[:, B:B + T], bias_f)
                nc.vector.tensor_sub(
                    rhs_c[:, B + T:B + 2 * T], bias_f, rhs_c[:, B:B + T]
                )
                # mask rows last: they overwrite the garbage the
                # activations left in rows 63/127 and zero those rows'
                # bias cols
                nc.scalar.dma_start(
                    out=rhs_c[RB - 1:RB, :], in_=maskrow[0:1, :]
                )
                nc.scalar.dma_start(
                    out=rhs_c[2 * RB - 1:2 * RB, :], in_=maskrow[1:2, :]
                )

            # ---- pair loop ----
            with (
                tc.tile_pool(name="psum_loop", bufs=3, space="PSUM") as psl,
                tc.tile_pool(name="o1", bufs=2) as o1p,
                tc.tile_pool(name="bp", bufs=3) as bpp,
            ):
                sums = work.tile([128, NT], F32)
                cnts = work.tile([128, NTC], F32)
                cntdst = work.tile([128, 2, B], BF16)
                o1 = None
                for tau in range(NT):
                    rep = psl.tile([128, WR], F32, tag="rep")
                    oh = t_oh[:, tau * 128:(tau + 1) * 128]
                    for n0, n1 in [(0, 512), (512, WR)]:
                        nc.tensor.matmul(
                            rep[:, n0:n1], oh, rhs_c[:, n0:n1],
                            start=True, stop=True,
                        )
                    bias_p = bpp.tile([128, 1], F32, tag="bias_p")
                    nc.vector.scalar_tensor_tensor(
                        out=cntdst[:, tau % 2, 0:2 * T],
                        in0=rep[:, B:B + 2 * T], scalar=1.0,
                        in1=t_sel[:, tau * 2 * T:(tau + 1) * 2 * T],
                        op0=mybir.AluOpType.mult, op1=mybir.AluOpType.mult,
                        accum_out=bias_p,
                    )
                    if tau % 2 == 0:
                        o1 = o1p.tile([128, 2, B], BF16, tag="o1")
                    nc.scalar.activation(
                        out=o1[:, tau % 2, :], in_=rep[:, 0:B],
                        func=mybir.ActivationFunctionType.Relu,
                        bias=bias_p, scale=-1.0,
                    )
                    if tau % 2 == 1 or tau == NT - 1:
                        j = tau // 2
                        k = tau % 2 + 1  # tiles in this batch
                        nc.vector.tensor_scalar(
                            out=cntdst[:, 0:k, :], in0=o1[:, 0:k, :],
                            scalar1=0.0, scalar2=0.0,
                            op0=mybir.AluOpType.not_equal,
                            op1=mybir.AluOpType.add,
                            accum_out=cnts[:, j:j + 1],
                        )
                        nc.vector.tensor_reduce(
                            out=sums[:, 2 * j:2 * j + k], in_=o1[:, 0:k, :],
                            axis=mybir.AxisListType.X, op=mybir.AluOpType.add,
                        )

                # ---- final reduce: free dim on DVE, partitions on gpsimd
                stat = work.tile([128, 2], F32)
                nc.vector.tensor_reduce(
                    out=stat[:, 0:1], in_=sums, axis=mybir.AxisListType.X,
                    op=mybir.AluOpType.add,
                )
                nc.vector.tensor_reduce(
                    out=stat[:, 1:2], in_=cnts, axis=mybir.AxisListType.X,
                    op=mybir.AluOpType.add,
                )
                ones_f = work.tile([128, 1], F32)
                nc.vector.memset(ones_f, 1.0)
                with tc.tile_pool(name="psum_out", bufs=1, space="PSUM") as pso:
                    pstat = pso.tile([1, 2], F32)
                    nc.tensor.matmul(pstat, ones_f, stat, start=True, stop=True)
                    ostat = work.tile([1, 2], F32)
                    nc.vector.tensor_copy(ostat, pstat)
                    nc.sync.dma_start(out=out_d[:, :], in_=ostat)

    _split_multi_waits(nc)
    return nc


def _schedule(labels: np.ndarray):
    """Group anchors by class, pair classes onto cores (big with small)."""
    vals, counts = np.unique(labels, return_counts=True)
    classes = [np.nonzero(labels == v)[0] for v in vals]
    order = np.argsort(-counts, kind="stable")
    classes = [classes[i] for i in order]
    sizes = [len(c) for c in classes]
    if len(classes) > 2 * N_CORES or max(sizes) > CMAX:
        return None  # device path infeasible for this label layout
    while len(classes) < 2 * N_CORES:
        classes.append(np.zeros((0,), dtype=np.int64))
    blocks = []
    for i in range(N_CORES):
        blocks.append((classes[i], classes[2 * N_CORES - 1 - i]))
    T = max(1, max(len(c) for c, _ in blocks))
    npairs = [len(a) * (len(a) - 1) + len(b) * (len(b) - 1) for a, b in blocks]
    NT = max(1, (max(npairs) + 127) // 128)
    return blocks, T, NT


def _host_fallback(X: np.ndarray, labels: np.ndarray) -> np.float32:
    """Exact numpy implementation (only for label layouts the device
    schedule cannot represent — cannot occur for randint(0,16) labels)."""
    Xd = X.astype(np.float64)
    dot = Xd @ Xd.T
    sq = np.diag(dot).copy()
    dm = np.maximum(sq[None, :] - 2.0 * dot + sq[:, None], 0.0)
    zero = dm == 0.0
    dm = np.sqrt(dm + zero * EPS) * (1.0 - zero)
    total = 0.0
    npos = 0
    B = len(labels)
    for i in range(B):
        pos = (labels == labels[i]) & (np.arange(B) != i)
        neg = labels != labels[i]
        p = dm[i, pos] + MARGIN
        n = dm[i, neg]
        tl = np.maximum(p[:, None] - n[None, :], 0.0)
        total += tl.sum()
        npos += (tl > EPS).sum()
    return np.float32(total / (npos + EPS))


def _make_in_maps(X: np.ndarray, lab: np.ndarray, blocks, T: int, NT: int):
    B, D = X.shape
    KC = D // 128
    W = 128 + B
    WR = B + 2 * T
    sq = (X.astype(np.float64) ** 2).sum(axis=1).astype(np.float32)

    in_maps = []
    for core in range(N_CORES):
        cls_a, cls_b = blocks[core]
        # anchor gather: class-a members in rows [0, CMAX), class-b in
        # [RB, RB+CMAX); pads duplicate the first member (their rows are
        # never selected by the 2-hot)
        row_idx = np.zeros(128, dtype=np.int64)
        fill = cls_a[0] if len(cls_a) else (cls_b[0] if len(cls_b) else 0)
        row_idx[:] = fill
        if len(cls_a):
            row_idx[0:len(cls_a)] = cls_a
        if len(cls_b):
            row_idx[RB:RB + len(cls_b)] = cls_b

        XT = np.ascontiguousarray(X.T)  # [D, B]
        comboW = np.empty((D, W), dtype=np.float32)
        comboW[:, 0:128] = XT[:, row_idx]
        comboW[:, 128:] = XT
        combo = np.ascontiguousarray(
            comboW.astype(NP_BF16).reshape(KC, 128, W).transpose(1, 0, 2)
        ).reshape(128, KC * W)

        halfW = np.empty((W,), dtype=np.float32)
        halfW[0:128] = -0.5 * sq[row_idx]
        halfW[128:] = -0.5 * sq
        hi = halfW.astype(NP_BF16)
        lo = (halfW - hi.astype(np.float32)).astype(NP_BF16)
        aug = np.stack([hi, lo])  # [2, W]

        sq_a = sq[row_idx].reshape(128, 1).astype(np.float32)

        # mask rows: -SHIFT on valid (other-class) columns, +BIG on own;
        # bias columns zero
        maskrow = np.zeros((2, WR), dtype=np.float32)
        for blk, cls in enumerate((cls_a, cls_b)):
            if len(cls):
                own = np.isin(lab, lab[cls[0]])
                maskrow[blk, 0:B] = np.where(own, BIG, -SHIFT)
            else:
                maskrow[blk, 0:B] = BIG  # empty class: kill everything

        # pair tables: 2-hot anchor+mask pick and bias-column select
        onehot = np.zeros((128, NT * 128), dtype=NP_BF16)
        selm = np.zeros((128, NT * 2 * T), dtype=NP_BF16)
        p = 0
        for blk, cls in enumerate((cls_a, cls_b)):
            m = len(cls)
            r0 = blk * RB
            for i in range(m):
                for t in range(m):
                    if t == i:
                        continue
                    tau, q = divmod(p, 128)
                    onehot[r0 + i, tau * 128 + q] = 1.0
                    onehot[r0 + CMAX, tau * 128 + q] = 1.0  # mask row
                    selm[q, tau * 2 * T + t] = 1.0
                    selm[q, tau * 2 * T + T + t] = 1.0
                    p += 1
        assert p <= NT * 128

        in_maps.append(
            {
                "combo": combo,
                "aug": aug,
                "sq_a": sq_a,
                "maskrow": maskrow.astype(NP_BF16),
                "onehot": onehot,
                "sel": selm,
            }
        )
    return in_maps


def kernel(embeddings: np.ndarray, labels: np.ndarray) -> np.ndarray:
    X = np.ascontiguousarray(np.asarray(embeddings), dtype=np.float32)
    lab = np.asarray(labels).astype(np.int64)
    B, D = X.shape
    assert B == B_TOTAL and D == D_MODEL, (B, D)

    sched = _schedule(lab)
    if sched is None:
        return _host_fallback(X, lab)
    blocks, T, NT = sched
    in_maps = _make_in_maps(X, lab, blocks, T, NT)

    key = (B, D, T, NT)
    nc = _PROGRAM_CACHE.get(key)
    if nc is None:
        nc = _build_program(B, D, T, NT)
        _PROGRAM_CACHE[key] = nc

    res = run_bass_kernel_spmd(nc, in_maps, core_ids=list(range(N_CORES)))
    total_sum = 0.0
    total_cnt = 0.0
    for r in res.results:
        o = np.asarray(r["out"], dtype=np.float64)
        total_sum += o[0, 0]
        total_cnt += o[0, 1]
    return np.float32(total_sum / (total_cnt + EPS))


# revision 16
# speedup vs baseline: 1.4485x; 1.4485x over previous
"""Batch-all triplet loss on 8 Trainium2 NeuronCores (Bass/Tile).

Math: with d = pairwise euclidean distance matrix of the B embeddings,
  loss = sum_{i,j,k valid} relu(d[i,j] - d[i,k] + margin) / (#positive + eps)
valid <=> i != j, labels[i] == labels[j], labels[i] != labels[k]
(the other distinctness constraints are implied by the label ones).

Sharding: anchors are grouped by class; each core hosts 2 classes in two
row blocks ([0:63) and [64:127)); rows 63/127 carry the per-class column
masks (-SHIFT on valid columns, +BIG on own-class ones), folded into the
replication matmul as a 2nd one in each stationary column (2-hot).

Device pipeline per core:
  warmup: ~14 dummy matmuls while input DMAs stream, so the PE's HAM
    governor ramps 1.2 -> 2.4 GHz before the real matmuls issue.
  prep: combo = [gathered anchors(128) | X^T(640)] bf16, K-packed.
    g = X_a.X^T - 0.5*sq_j (bf16 K-chunks + a K=2 hi/lo norm row pair)
    -> ACT Relu(-2g + sq_a) -> ACT Sqrt -> rhs_c[:, 0:B] (bf16 distances).
    Mask rows 63/127 (incl. zeroed bias cols) come from a tiny host DMA.
    A partner matmul against the anchor block gives bias hi/lo bf16 cols
    rhs_c[:, B:B+2T] (bias = d_pos + margin - SHIFT, hi/lo split keeps the
    pair threshold ~f32 exact).
  pair loop, NT tiles of 128 (anchor,positive) pairs:
  - PE 2-hot matmul (anchor row + its class's mask row) replicates
    rep[p, :] = d_masked[anchor(p), :] | bias cols, into PSUM.
  - DVE stt: sel one-hot picks the pair's own bias -> bias_p [128,1] f32.
  - ACT: o1 = Relu(-rep + bias_p) -> SBUF bf16 (the relu'd triplet values).
  - every 2 tiles, DVE on the packed bf16 o1 (4x perf mode):
    tensor_scalar(not_equal 0, accum) -> positive count;
    tensor_reduce(add) -> per-tile sums.
  - invalid negatives see +BIG (contribute 0); padded pairs have all-zero
    rows and zero bias (contribute 0 to both sums).
  tail: free-dim reduce -> [128, 2], gpsimd partition_all_reduce -> one
    [1, 2] (sum, count) DMA'd out; host combines the 8 cores.
"""

import numpy as np

import bass_rust
import concourse.bass as bass
import concourse.tile as tile
from concourse import mybir
from concourse import bass_isa
from concourse.bass_utils import run_bass_kernel_spmd

N_CORES = 8
D_MODEL = 512
B_TOTAL = 640
MARGIN = 0.3
EPS = 1e-8
RB = 64          # rows per class block; rows RB-1 / 2*RB-1 are mask rows
CMAX = RB - 1    # max class size the device path supports
BIG = 1.0e30
SHIFT = 32.0     # d values live in ~[27.6, 37]; recentring helps bf16
F32 = mybir.dt.float32
BF16 = mybir.dt.bfloat16
NP_BF16 = mybir.dt.np(mybir.dt.bfloat16)

_PROGRAM_CACHE: dict = {}


def _split_multi_waits(nc):
    """This toolchain's walrus codegen supports only ONE sync-wait per
    instruction; Tile can emit several. Move the extra waits onto
    same-engine NoOps inserted immediately before the instruction."""
    for func in nc.m.functions:
        for block in func.blocks:
            out = []
            for inst in block.instructions:
                si = inst.sync_info
                waits = list(si.on_wait) if si else []
                if len(waits) > 1:
                    for j, w in enumerate(waits[:-1]):
                        nop = mybir.InstNoOp(
                            name=f"{inst.name}-wsplit{j}", ins=[], outs=[]
                        )
                        nop.engine = inst.engine
                        nop.sync_info = bass_rust.SyncInfo(on_wait=[w], on_update=[])
                        out.append(nop)
                    inst.sync_info = bass_rust.SyncInfo(
                        on_wait=[waits[-1]], on_update=list(si.on_update)
                    )
                out.append(inst)
            block.instructions = out


def _build_program(B: int, D: int, T: int, NT: int):
    """One SPMD program for all 8 cores; per-core behavior is data-driven."""
    nc = bass.Bass()

    KC = D // 128     # K chunks of the matmul contraction
    W = 128 + B       # combo cols: [0:128) gathered anchors, [128:) X^T
    WR = B + 2 * T    # replicated tile: negatives + bias hi + lo columns

    # packed [128, KC, W]: partition p, chunk c -> K-row c*128+p
    combo = nc.declare_dram_parameter("combo", [128, KC * W], BF16, isOutput=False)
    aug = nc.declare_dram_parameter("aug", [2, W], BF16, isOutput=False)
    sq_a = nc.declare_dram_parameter("sq_a", [128, 1], F32, isOutput=False)
    maskrow = nc.declare_dram_parameter("maskrow", [2, WR], BF16, isOutput=False)
    onehot = nc.declare_dram_parameter("onehot", [128, NT * 128], BF16, isOutput=False)
    sel = nc.declare_dram_parameter("sel", [128, NT * 2 * T], BF16, isOutput=False)
    out_d = nc.declare_dram_parameter("out", [1, 2], F32, isOutput=True)

    NTC = (NT + 1) // 2  # count/reduce instruction pairs (2 tiles each)

    with tile.TileContext(nc) as tc:
        with (
            tc.tile_pool(name="const", bufs=1) as const,
            tc.tile_pool(name="work", bufs=1) as work,
        ):
            # ---- input DMAs first (combo is the critical path), spread
            # across the sync and scalar queues ----------------------------
            ktile = const.tile([128, KC, W], BF16)
            combo3 = combo[:, :].rearrange("p (c w) -> p c w", c=KC)
            for ki in range(KC):
                eng = nc.sync if ki % 2 == 0 else nc.scalar
                eng.dma_start(out=ktile[:, ki, :], in_=combo3[:, ki, :])
            taug = const.tile([2, W], BF16)
            nc.sync.dma_start(out=taug, in_=aug[:, :])
            sqa = const.tile([128, 1], F32)
            nc.sync.dma_start(out=sqa, in_=sq_a[:, :])
            # oh/sel interleaved in 3 chunks each so early tiles unblock
            t_oh = const.tile([128, NT * 128], BF16)
            t_sel = const.tile([128, NT * 2 * T], BF16)
            tchunks = [(NT * i // 3, NT * (i + 1) // 3) for i in range(3)]
            for t0, t1 in tchunks:
                nc.gpsimd.dma_start(
                    out=t_oh[:, t0 * 128:t1 * 128],
                    in_=onehot[:, t0 * 128:t1 * 128],
                )
                nc.gpsimd.dma_start(
                    out=t_sel[:, t0 * 2 * T:t1 * 2 * T],
                    in_=sel[:, t0 * 2 * T:t1 * 2 * T],
                )

            # ---- PE warmup source + ACT table preload -------------------
            warm_src = const.tile([128, 128], BF16)
            nc.vector.memset(warm_src, 0.0)
            warm = const.tile([1, 8], F32)
            nc.vector.memset(warm, 1.0)
            nc.scalar.activation(out=warm, in_=warm,
                                 func=mybir.ActivationFunctionType.Relu)
            nc.scalar.activation(out=warm, in_=warm,
                                 func=mybir.ActivationFunctionType.Sqrt)

            ones2 = const.tile([2, 128], BF16)
            nc.vector.memset(ones2, 1.0)

            rhs_c = work.tile([128, WR], BF16)  # [d_in | bias hi | bias lo]

            with tc.tile_pool(name="psum_warm", bufs=1, space="PSUM") as psw:
                wp = psw.tile([128, 128], F32)
                for _ in range(12):
                    nc.tensor.matmul(wp, warm_src, warm_src,
                                     start=True, stop=True)

            with tc.tile_pool(name="psum_prep", bufs=1, space="PSUM") as psp:
                # ---- distance rows: PSUM g = dot(x_a, x_j) - 0.5*sq_j ----
                # g col j <-> combo col 128+j
                g = psp.tile([128, B], F32)
                for n0, n1 in [(0, 512), (512, B)]:
                    for ki in range(KC):
                        nc.tensor.matmul(
                            g[:, n0:n1],
                            ktile[:, ki, 0:128],
                            ktile[:, ki, 128 + n0:128 + n1],
                            start=(ki == 0),
                            stop=False,
                        )
                    nc.tensor.matmul(
                        g[:, n0:n1], ones2, taug[:, 128 + n0:128 + n1],
                        start=False, stop=True,
                    )
                dsq = work.tile([128, B], F32)
                nc.scalar.activation(
                    out=dsq, in_=g, func=mybir.ActivationFunctionType.Relu,
                    bias=sqa, scale=-2.0,
                )
                nc.scalar.activation(
                    out=rhs_c[:, 0:B], in_=dsq,
                    func=mybir.ActivationFunctionType.Sqrt,
                )

                # ---- positive-pair distances: bias[r, t] -----------------
                pb = psp.tile([128, T], F32)
                for blk in range(2):
                    r0, r1 = blk * RB, (blk + 1) * RB
                    for ki in range(KC):
                        nc.tensor.matmul(
                            pb[r0:r1, :],
                            ktile[:, ki, r0:r1],
                            ktile[:, ki, r0:r0 + T],
                            start=(ki == 0),
                            stop=False,
                        )
                    nc.tensor.matmul(
                        pb[r0:r1, :], ones2[:, r0:r1], taug[:, r0:r0 + T],
                        start=False, stop=True,
                    )
                bsq = work.tile([128, T], F32)
                nc.scalar.activation(
                    out=bsq, in_=pb, func=mybir.ActivationFunctionType.Relu,
                    bias=sqa, scale=-2.0,
                )
                bd = work.tile([128, T], F32)
                nc.scalar.activation(
                    out=bd, in_=bsq, func=mybir.ActivationFunctionType.Sqrt,
                )
                # bias = d_pos + margin - 32, carried as bf16 hi + lo so
                # the pair threshold stays ~f32 exact
                bias_f = work.tile([128, T], F32)
                nc.vector.tensor_scalar(
                    out=bias_f, in0=bd,
                    scalar1=float(MARGIN - SHIFT), scalar2=None,
                    op0=mybir.AluOpType.add,
                )
                nc.vector.tensor_copy(rhs_c[:, B:B + T], bias_f)
                nc.vector.tensor_sub(
                    rhs_c[:, B + T:B + 2 * T], bias_f, rhs_c[:, B:B + T]
                )
                # mask rows last: they overwrite the garbage the
                # activations left in rows 63/127 and zero those rows'
                # bias cols
                nc.scalar.dma_start(
                    out=rhs_c[RB - 1:RB, :], in_=maskrow[0:1, :]
                )
                nc.scalar.dma_start(
                    out=rhs_c[2 * RB - 1:2 * RB, :], in_=maskrow[1:2, :]
                )

            # ---- pair loop ----
            NP = (NT + 1) // 2       # tile pairs
            NQ = (NP + 1) // 2       # count groups (4 tiles / group)
            NGRP = (NT + 3) // 4  # count groups (<=4 tiles each)
            with (
                tc.tile_pool(name="psum_loop", bufs=3, space="PSUM") as psl,
                tc.tile_pool(name="psum_acc", bufs=1, space="PSUM") as psa,
                tc.tile_pool(name="o1", bufs=2) as o1p,
                tc.tile_pool(name="c2", bufs=2) as c2p,
                tc.tile_pool(name="c4", bufs=2) as c4p,
                tc.tile_pool(name="bp", bufs=3) as bpp,
            ):
                sums = work.tile([128, NT], F32)
                csum = psa.tile([1, 512], F32)
                ones_b = const.tile([128, 1], BF16)
                nc.vector.memset(ones_b, 1.0)
                sttjunk = work.tile([128, 2 * T], BF16)
                o1 = None
                c2 = None
                ngrp = 0
                for tau in range(NT):
                    rep = psl.tile([128, WR], F32, tag="rep")
                    oh = t_oh[:, tau * 128:(tau + 1) * 128]
                    for n0, n1 in [(0, 512), (512, WR)]:
                        nc.tensor.matmul(
                            rep[:, n0:n1], oh, rhs_c[:, n0:n1],
                            start=True, stop=True,
                        )
                    bias_p = bpp.tile([128, 1], F32, tag="bias_p")
                    nc.vector.scalar_tensor_tensor(
                        out=sttjunk,
                        in0=rep[:, B:B + 2 * T], scalar=1.0,
                        in1=t_sel[:, tau * 2 * T:(tau + 1) * 2 * T],
                        op0=mybir.AluOpType.mult, op1=mybir.AluOpType.mult,
                        accum_out=bias_p,
                    )
                    if tau % 2 == 0:
                        o1 = o1p.tile([128, 2, B], BF16, tag="o1")
                    nc.scalar.activation(
                        out=o1[:, tau % 2, :], in_=rep[:, 0:B],
                        func=mybir.ActivationFunctionType.Relu,
                        bias=bias_p, scale=-1.0,
                        accum_out=sums[:, tau:tau + 1],
                    )
                    if tau % 2 == 1 or tau == NT - 1:
                        p = tau // 2
                        k = tau % 2 + 1  # tiles in this batch
                        if p % 2 == 0:
                            c2 = c2p.tile([128, 2, B], BF16, tag="c2")
                        # 0/1 indicators (bf16 elementwise, fast mode)
                        cnt01 = o1p.tile([128, 2, B], BF16, tag="cnt01")
                        nc.vector.tensor_scalar(
                            out=cnt01[:, 0:k, :], in0=o1[:, 0:k, :],
                            scalar1=0.0, scalar2=None,
                            op0=mybir.AluOpType.not_equal,
                        )
                        if k == 2:
                            nc.vector.tensor_add(
                                c2[:, p % 2, :], cnt01[:, 0, :], cnt01[:, 1, :]
                            )
                        else:
                            nc.vector.tensor_copy(c2[:, p % 2, :], cnt01[:, 0, :])
                        if p % 2 == 1 or tau == NT - 1:
                            if p % 2 == 1:
                                c4 = c4p.tile([128, B], BF16, tag="c4")
                                nc.vector.tensor_add(
                                    c4, c2[:, 0, :], c2[:, 1, :]
                                )
                                src = c4
                            else:
                                src = c2[:, 0, :]
                            # fold cols [512:640] into [0:128] (counts stay
                            # small exact ints in bf16), then partition-
                            # reduce on the tensor engine, accumulating
                            # across groups in one PSUM bank
                            c4f = c4p.tile([128, 512], BF16, tag="c4f")
                            nc.vector.tensor_add(
                                c4f[:, 0:128], src[:, 0:128], src[:, 512:B]
                            )
                            nc.vector.tensor_copy(
                                c4f[:, 128:512], src[:, 128:512]
                            )
                            nc.tensor.matmul(
                                csum, ones_b, c4f,
                                start=(ngrp == 0),
                                stop=(ngrp == NGRP - 1),
                                skip_group_check=True,
                            )
                            ngrp += 1

                # ---- final reduce: partitions via a ones-column fp32
                # matmul, then free-dim on DVE
                ones_f = work.tile([128, 1], F32)
                nc.vector.memset(ones_f, 1.0)
                pmm = psa.tile([1, NT], F32)
                nc.tensor.matmul(pmm, ones_f, sums, start=True, stop=True)
                ostat = work.tile([1, 2], F32)
                nc.vector.tensor_reduce(
                    out=ostat[:, 0:1], in_=pmm, axis=mybir.AxisListType.X,
                    op=mybir.AluOpType.add,
                )
                nc.vector.tensor_reduce(
                    out=ostat[:, 1:2], in_=csum, axis=mybir.AxisListType.X,
                    op=mybir.AluOpType.add,
                )
                nc.sync.dma_start(out=out_d[:, :], in_=ostat)

    _split_multi_waits(nc)
    return nc


def _schedule(labels: np.ndarray):
    """Group anchors by class, pair classes onto cores (big with small)."""
    vals, counts = np.unique(labels, return_counts=True)
    classes = [np.nonzero(labels == v)[0] for v in vals]
    order = np.argsort(-counts, kind="stable")
    classes = [classes[i] for i in order]
    sizes = [len(c) for c in classes]
    if len(classes) > 2 * N_CORES or max(sizes) > CMAX:
        return None  # device path infeasible for this label layout
    while len(classes) < 2 * N_CORES:
        classes.append(np.zeros((0,), dtype=np.int64))
    blocks = []
    for i in range(N_CORES):
        blocks.append((classes[i], classes[2 * N_CORES - 1 - i]))
    T = max(1, max(len(c) for c, _ in blocks))
    npairs = [len(a) * (len(a) - 1) + len(b) * (len(b) - 1) for a, b in blocks]
    NT = max(1, (max(npairs) + 127) // 128)
    return blocks, T, NT


def _host_fallback(X: np.ndarray, labels: np.ndarray) -> np.float32:
    """Exact numpy implementation (only for label layouts the device
    schedule cannot represent — cannot occur for randint(0,16) labels)."""
    Xd = X.astype(np.float64)
    dot = Xd @ Xd.T
    sq = np.diag(dot).copy()
    dm = np.maximum(sq[None, :] - 2.0 * dot + sq[:, None], 0.0)
    zero = dm == 0.0
    dm = np.sqrt(dm + zero * EPS) * (1.0 - zero)
    total = 0.0
    npos = 0
    B = len(labels)
    for i in range(B):
        pos = (labels == labels[i]) & (np.arange(B) != i)
        neg = labels != labels[i]
        p = dm[i, pos] + MARGIN
        n = dm[i, neg]
        tl = np.maximum(p[:, None] - n[None, :], 0.0)
        total += tl.sum()
        npos += (tl > EPS).sum()
    return np.float32(total / (npos + EPS))


def _make_in_maps(X: np.ndarray, lab: np.ndarray, blocks, T: int, NT: int):
    B, D = X.shape
    KC = D // 128
    W = 128 + B
    WR = B + 2 * T
    sq = (X.astype(np.float64) ** 2).sum(axis=1).astype(np.float32)

    in_maps = []
    for core in range(N_CORES):
        cls_a, cls_b = blocks[core]
        # anchor gather: class-a members in rows [0, CMAX), class-b in
        # [RB, RB+CMAX); pads duplicate the first member (their rows are
        # never selected by the 2-hot)
        row_idx = np.zeros(128, dtype=np.int64)
        fill = cls_a[0] if len(cls_a) else (cls_b[0] if len(cls_b) else 0)
        row_idx[:] = fill
        if len(cls_a):
            row_idx[0:len(cls_a)] = cls_a
        if len(cls_b):
            row_idx[RB:RB + len(cls_b)] = cls_b

        XT = np.ascontiguousarray(X.T)  # [D, B]
        comboW = np.empty((D, W), dtype=np.float32)
        comboW[:, 0:128] = XT[:, row_idx]
        comboW[:, 128:] = XT
        combo = np.ascontiguousarray(
            comboW.astype(NP_BF16).reshape(KC, 128, W).transpose(1, 0, 2)
        ).reshape(128, KC * W)

        halfW = np.empty((W,), dtype=np.float32)
        halfW[0:128] = -0.5 * sq[row_idx]
        halfW[128:] = -0.5 * sq
        hi = halfW.astype(NP_BF16)
        lo = (halfW - hi.astype(np.float32)).astype(NP_BF16)
        aug = np.stack([hi, lo])  # [2, W]

        sq_a = sq[row_idx].reshape(128, 1).astype(np.float32)

        # mask rows: -SHIFT on valid (other-class) columns, +BIG on own;
        # bias columns zero
        maskrow = np.zeros((2, WR), dtype=np.float32)
        for blk, cls in enumerate((cls_a, cls_b)):
            if len(cls):
                own = np.isin(lab, lab[cls[0]])
                maskrow[blk, 0:B] = np.where(own, BIG, -SHIFT)
            else:
                maskrow[blk, 0:B] = BIG  # empty class: kill everything

        # pair tables: 2-hot anchor+mask pick and bias-column select
        onehot = np.zeros((128, NT * 128), dtype=NP_BF16)
        selm = np.zeros((128, NT * 2 * T), dtype=NP_BF16)
        p = 0
        for blk, cls in enumerate((cls_a, cls_b)):
            m = len(cls)
            r0 = blk * RB
            for i in range(m):
                for t in range(m):
                    if t == i:
                        continue
                    tau, q = divmod(p, 128)
                    onehot[r0 + i, tau * 128 + q] = 1.0
                    onehot[r0 + CMAX, tau * 128 + q] = 1.0  # mask row
                    selm[q, tau * 2 * T + t] = 1.0
                    selm[q, tau * 2 * T + T + t] = 1.0
                    p += 1
        assert p <= NT * 128

        in_maps.append(
            {
                "combo": combo,
                "aug": aug,
                "sq_a": sq_a,
                "maskrow": maskrow.astype(NP_BF16),
                "onehot": onehot,
                "sel": selm,
            }
        )
    return in_maps


def kernel(embeddings: np.ndarray, labels: np.ndarray) -> np.ndarray:
    X = np.ascontiguousarray(np.asarray(embeddings), dtype=np.float32)
    lab = np.asarray(labels).astype(np.int64)
    B, D = X.shape
    assert B == B_TOTAL and D == D_MODEL, (B, D)

    sched = _schedule(lab)
    if sched is None:
        return _host_fallback(X, lab)
    blocks, T, NT = sched
    in_maps = _make_in_maps(X, lab, blocks, T, NT)

    key = (B, D, T, NT)
    nc = _PROGRAM_CACHE.get(key)
    if nc is None:
        nc = _build_program(B, D, T, NT)
        _PROGRAM_CACHE[key] = nc

    res = run_bass_kernel_spmd(nc, in_maps, core_ids=list(range(N_CORES)))
    total_sum = 0.0
    total_cnt = 0.0
    for r in res.results:
        o = np.asarray(r["out"], dtype=np.float64)
        total_sum += o[0, 0]
        total_cnt += o[0, 1]
    return np.float32(total_sum / (total_cnt + EPS))
